# revision 32
# baseline (speedup 1.0000x reference)
"""Causal multi-head attention block (QKV proj -> attention -> out proj) on 8
Trainium2 NeuronCores.

Sharding: core i handles batch b = i//2 and head-group g = i%2 (6 of 12 heads).
Each core computes its heads' attention output and a partial output projection
(rows g*384:(g+1)*384 of w_proj); the host sums the two partials per batch and
adds b_proj.

On-core dataflow (per core):
  x^T tiles  [c,t]   via DMA-transpose (bf16), chunked per 512-token strip so
                     strip-0 compute starts as soon as 1/4 of x^T has landed
  q^T, k^T   [n,t]   = w-stationary matmuls, pair-stacked 2 heads/tile
  v          [t,n]   natural layout, with a ones column appended per head
  S^T        [kt,qt] = k^T-stationary matmul, row-packed pairs (K=64 halves);
                     diagonal blocks get the -1e9 triangular mask added inside
                     the PSUM accumulation group via an extra I@tri matmul
  P^T        = exp(S^T)  (no max subtraction: |scores| is O(10), safe in fp32)
  [out^T; l] [65,qt] = [v|1]-stationary matmul per head (l = softmax denom)
  out^T * (1/l)      -> ao^T [f,t], fed as lhsT to the projection matmul

The attention stream is software-pipelined: scores for group g+1 are emitted
before exp(g)+AV(g), so the PE never sits in front of a not-yet-finished exp.
PE-heavy b-units (next pair's QKV matmuls, final projection) are interleaved
between the attention a-units: the attention phase is ScalarE(exp)-bound, PE
executes in program order, and any PE idle window >3.4us makes the HAM clock
gate re-throttle the PE to 1.2 GHz, doubling every matmul's cost.
"""

import math
from contextlib import ExitStack

import numpy as np
import ml_dtypes

import concourse.bass as bass
import concourse.mybir as mybir
import concourse.tile as tile
from concourse import bacc, library_config
from concourse.bass_utils import run_bass_kernel_spmd

B, T_FULL, C = 4, 2048, 768
NH, HD = 12, 64
HL = NH // 2            # heads per core
NPAIR = HL // 2         # head pairs per core
NQK = HL * HD           # 384 features per core for each of q/k/v
N_CORES = 8
P = 128
SW = 512                # qt strip width
NC_T = C // P           # 6 contraction tiles
F32 = mybir.dt.float32
BF16 = mybir.dt.bfloat16
NPF = np.float32
NPBF = ml_dtypes.bfloat16

_CACHE: dict = {}


def build(T: int = T_FULL, interleave_on: bool = True, warmup: int = 56):
    NT = T // P
    NSTRIP = T // SW
    nc = bacc.Bacc("TRN2", target_bir_lowering=False, debug=False,
                   num_devices=N_CORES)
    xt_d = nc.dram_tensor("xt", [C, T], BF16, kind="ExternalInput")
    w_d = nc.dram_tensor("wqkv", [C, 3 * NQK], BF16, kind="ExternalInput")
    bqk_d = nc.dram_tensor("bqk", [P, 2 * NPAIR], F32, kind="ExternalInput")
    bv_d = nc.dram_tensor("bv", [1, NQK], F32, kind="ExternalInput")
    wp_d = nc.dram_tensor("wp", [NQK, C], BF16, kind="ExternalInput")
    tri_d = nc.dram_tensor("tri", [P, P], BF16, kind="ExternalInput")
    idn_d = nc.dram_tensor("idn", [P, P], BF16, kind="ExternalInput")
    out_d = nc.dram_tensor("out", [T, C], BF16, kind="ExternalOutput")

    EXP = mybir.ActivationFunctionType.Exp
    ADD = mybir.AluOpType.add
    MUL = mybir.AluOpType.mult

    with ExitStack() as ctx:
        tc = ctx.enter_context(tile.TileContext(nc))
        persist = ctx.enter_context(tc.tile_pool(name="persist", bufs=1))
        ppool = ctx.enter_context(tc.tile_pool(name="pt", bufs=6))
        smallp = ctx.enter_context(tc.tile_pool(name="small", bufs=4))
        outp = ctx.enter_context(tc.tile_pool(name="outsb", bufs=3))
        ps_s = ctx.enter_context(tc.tile_pool(name="ps_s", bufs=2, space="PSUM"))
        ps_q = ctx.enter_context(tc.tile_pool(name="ps_q", bufs=2, space="PSUM"))
        ps_av = ctx.enter_context(tc.tile_pool(name="ps_av", bufs=2, space="PSUM"))

        nc.gpsimd.load_library(library_config.attn)

        # ---- persistent inputs ----
        # Small tensors first (tri gates the warmup matmuls), then w q/k
        # blocks for pair 0, then strip-chunked x^T transposes ordered so
        # strip 0 lands first. Transposes only run on the two HWDGE queues
        # (sync, scalar); weight blocks are interleaved between them.
        # tiny tensors: declared here, issued on the HWDGE queues right after
        # the strip-0/pair-0 loads (the gpsimd SWDGE queue only starts at
        # ~22us, far too late for tri/bqk which gate the first attention)
        tri_sb = persist.tile([P, P], BF16)
        idn_sb = persist.tile([P, P], BF16)
        bqk_sb = persist.tile([P, 2 * NPAIR], F32)
        bv_bc = persist.tile([P, NQK], F32)
        bv_ap = bv_d.ap()
        bv_bcast = bass.AP(tensor=bv_ap.tensor, offset=bv_ap.offset,
                           ap=[[0, P], [1, NQK]])

        w_sb = persist.tile([P, NC_T, 3 * NQK], BF16)
        w_re = w_d.ap().rearrange("(a p) n -> p a n", p=P)

        def w_block(eng, b):
            eng.dma_start(w_sb[:, :, b * P:(b + 1) * P],
                          w_re[:, :, b * P:(b + 1) * P])

        # x is transposed to [C, T] on the host, so x^T loads are fast
        # linear DMAs instead of 256B-packet DMA transposes (which starved
        # the PE for the first ~60us). Chunked per strip, split across both
        # HWDGE queues so strip-0 consumers start early.
        xT = persist.tile([P, NC_T, T], BF16)
        xt_re = xt_d.ap().rearrange("(a p) t -> p a t", p=P)

        def x_chunk(eng, s, clo, chi):
            eng.dma_start(xT[:, clo:chi, s * SW:(s + 1) * SW],
                          xt_re[:, clo:chi, s * SW:(s + 1) * SW])

        # strip-0 x chunks lead both HWDGE queues; weight blocks and the
        # small tensors interleave behind them in need-order
        for s in range(NSTRIP):
            x_chunk(nc.sync, s, 0, NC_T // 2)
            x_chunk(nc.scalar, s, NC_T // 2, NC_T)
            if s == 0:
                w_block(nc.scalar, 0)              # q pair 0
                w_block(nc.sync, NPAIR)            # k pair 0
                nc.sync.dma_start(bqk_sb[:], bqk_d.ap())
                nc.scalar.dma_start(tri_sb[:], tri_d.ap())
                nc.scalar.dma_start(idn_sb[:], idn_d.ap())
                # one DMA covers v weights for all 3 pairs (contiguous cols)
                nc.sync.dma_start(w_sb[:, :, 2 * NQK:3 * NQK],
                                  w_re[:, :, 2 * NQK:3 * NQK])
                nc.sync.dma_start(bv_bc[:], bv_bcast)
            elif s == 1:
                w_block(nc.scalar, 1)              # q pair 1
                w_block(nc.sync, NPAIR + 1)        # k pair 1
        # remaining weight blocks + wp
        w_block(nc.scalar, 2)                # q pair 2
        w_block(nc.scalar, NPAIR + 2)        # k pair 2
        wp_sb = persist.tile([P, NQK // P, C], BF16)
        nc.sync.dma_start(wp_sb[:], wp_d.ap().rearrange("(a p) n -> p a n", p=P))

        # HAM warmup: keep PE busy with throwaway matmuls while x^T streams
        # in; a memset tile avoids waiting for any DMA at all
        warm_w = persist.tile([P, P], BF16)
        nc.vector.memset(warm_w[:], 0.0)
        warm_ps = ps_q.tile([P, P], F32, tag="q")
        for i in range(warmup):
            nc.tensor.matmul(warm_ps[:], warm_w[:], warm_w[:],
                             start=(i == 0), stop=(i == warmup - 1),
                             skip_group_check=True)

        # per-pair tensors (separate tiles so pair p+1 writes don't create
        # false deps against pair p reads)
        qT = [persist.tile([P, T], BF16, name=f"qT{i}", tag=f"qT{i}") for i in range(NPAIR)]
        # k^T stored zero-padded per head: kTZ[:, h] = [K_A; 0] (h=0) or
        # [0; K_B] (h=1). A K=64 matmul streams at HALF the column rate of
        # K=128 (hw-measured), so scores use the padded K=128 form against
        # the full 2-head qT: [K_A; 0]^T [Q_A; Q_B] = S_A at full rate.
        kTZ = [persist.tile([P, 2, T], BF16, name=f"kTZ{i}", tag=f"kTZ{i}")
               for i in range(NPAIR)]
        v_sb = [persist.tile([P, NT, 2, HD + 1], BF16, name=f"v{i}", tag=f"v{i}")
                for i in range(NPAIR)]
        aoT = [persist.tile([P, T], BF16, name=f"aoT{i}", tag=f"aoT{i}") for i in range(NPAIR)]
        for p in range(NPAIR):
            nc.vector.memset(v_sb[p][:, :, :, HD:HD + 1], 1.0)
            nc.vector.memset(kTZ[p][64:P, 0, :], 0.0)
            nc.vector.memset(kTZ[p][0:64, 1, :], 0.0)

        def qk_unit(p, is_k, s):
            bidx = NPAIR + p if is_k else p
            fi = bidx * P
            ps_t = ps_q.tile([P, SW], F32, tag="q")
            for cb in range(NC_T):
                nc.tensor.matmul(
                    ps_t[:], w_sb[:, cb, fi:fi + P],
                    xT[:, cb, s * SW:(s + 1) * SW],
                    start=(cb == 0), stop=(cb == NC_T - 1))
            ss = slice(s * SW, (s + 1) * SW)
            if is_k:
                nc.vector.tensor_scalar_add(
                    kTZ[p][0:64, 0, ss], ps_t[0:64, :],
                    bqk_sb[0:64, bidx:bidx + 1])
                nc.vector.tensor_scalar_add(
                    kTZ[p][64:P, 1, ss], ps_t[64:P, :],
                    bqk_sb[64:P, bidx:bidx + 1])
            else:
                nc.vector.tensor_scalar_add(
                    qT[p][:, ss], ps_t[:], bqk_sb[:, bidx:bidx + 1])

        def v_unit(tt):
            # v for all 3 pairs in one 384-wide matmul group (1/3 the
            # instruction+ldweights count of per-pair 128-wide groups)
            ps_t = ps_q.tile([P, NPAIR * P], F32, tag="q")
            vcols = 2 * NQK
            for cb in range(NC_T):
                nc.tensor.matmul(
                    ps_t[:], xT[:, cb, tt * P:(tt + 1) * P],
                    w_sb[:, cb, vcols:vcols + NPAIR * P],
                    start=(cb == 0), stop=(cb == NC_T - 1))
            for p in range(NPAIR):
                nc.vector.tensor_tensor(
                    out=v_sb[p][:, tt, :, 0:HD], in0=ps_t[:, p * P:(p + 1) * P],
                    in1=bv_bc[:, p * P:(p + 1) * P], op=ADD)

        def qk_units(p):
            us = []
            for is_k in (False, True):
                for s in range(NSTRIP):
                    us.append(lambda k=is_k, s=s: qk_unit(p, k, s))
            return us

        def proj_unit(tt):
            tts = slice(tt * P, (tt + 1) * P)
            ot = outp.tile([P, C], BF16)
            for nch, n0, n1 in ((0, 0, SW), (1, SW, C)):
                pr = ps_q.tile([P, SW], F32, tag="q")
                for ft in range(NQK // P):
                    nc.tensor.matmul(pr[:, 0:n1 - n0], aoT[ft][:, tts],
                                     wp_sb[:, ft, n0:n1],
                                     start=(ft == 0), stop=(ft == NQK // P - 1))
                nc.vector.tensor_copy(ot[:, n0:n1], pr[:, 0:n1 - n0])
            nc.sync.dma_start(out_d.ap()[tts, :], ot[:])

        def attn_units(p):
            """a-units for pair p's attention, software-pipelined: scores for
            group g+1 (and the first 2 groups of the next strip) are separate
            units emitted before exp(g)+AV(g), so b-units land between the
            score matmuls and the exp-gated AV matmuls."""
            us = []
            marks = {}
            sgrp = {}
            avst = {}

            def S_unit(s, g):
                def run():
                    sA = ps_s.tile([P, 2, SW], F32, tag="s", name="sA")
                    sB = ps_s.tile([P, 2, SW], F32, tag="s", name="sB")
                    sgrp[(s, g)] = (sA, sB)
                    for sub in range(2):
                        kt = 2 * g + sub
                        j = kt - 4 * s
                        c0 = max(j, 0) * P
                        kts = slice(kt * P, (kt + 1) * P)
                        qts = slice(s * SW + c0, (s + 1) * SW)
                        diag = j >= 0
                        nc.tensor.matmul(sA[:, sub, c0:SW], kTZ[p][:, 0, kts],
                                         qT[p][:, qts], start=True,
                                         stop=not diag)
                        nc.tensor.matmul(sB[:, sub, c0:SW], kTZ[p][:, 1, kts],
                                         qT[p][:, qts], start=True,
                                         stop=not diag)
                        if diag:  # -1e9 tri mask accumulated on the PE
                            nc.tensor.matmul(sA[:, sub, c0:c0 + P], idn_sb[:],
                                             tri_sb[:], start=False, stop=True,
                                             skip_group_check=True)
                            nc.tensor.matmul(sB[:, sub, c0:c0 + P], idn_sb[:],
                                             tri_sb[:], start=False, stop=True,
                                             skip_group_check=True)
                return run

            def EA_unit(s, g, n_kt):
                def run():
                    if g == 0:
                        avst[s] = (ps_av.tile([P, SW], F32, name="avA", tag="av"),
                                   ps_av.tile([P, SW], F32, name="avB", tag="av"))
                    avA, avB = avst[s]
                    sA, sB = sgrp.pop((s, g))
                    c0m = max(2 * g - 4 * s, 0) * P
                    pA = ppool.tile([P, 2, SW], BF16, tag="pt")
                    pB = ppool.tile([P, 2, SW], BF16, tag="pt")
                    nc.scalar.activation(pA[:, :, c0m:], sA[:, :, c0m:], EXP)
                    nc.scalar.activation(pB[:, :, c0m:], sB[:, :, c0m:], EXP)
                    for sub in range(2):
                        kt = 2 * g + sub
                        c0 = max(kt - 4 * s, 0) * P
                        first, last = kt == 0, kt == n_kt - 1
                        nc.tensor.matmul(
                            avA[0:HD + 1, c0:SW], v_sb[p][:, kt, 0, :],
                            pA[:, sub, c0:SW], start=first, stop=last,
                            skip_group_check=True)
                        nc.tensor.matmul(
                            avB[0:HD + 1, c0:SW], v_sb[p][:, kt, 1, :],
                            pB[:, sub, c0:SW], start=first, stop=last,
                            skip_group_check=True)
                return run

            def norm_unit(s):
                def run():
                    if p == NPAIR - 1 and s == NSTRIP - 1:
                        # the very last norm has no b-units left to keep the
                        # PE busy: emit throwaway matmuls so HAM stays warm
                        # and the final proj units run at full clock
                        jk = ps_q.tile([P, SW], F32, tag="q")
                        for i in range(22):
                            nc.tensor.matmul(jk[:], warm_w[:],
                                             xT[:, 0, 0:SW],
                                             start=(i == 0), stop=(i == 21),
                                             skip_group_check=True)
                    avA, avB = avst.pop(s)
                    lA = smallp.tile([1, SW], F32, tag="lrow")
                    lB = smallp.tile([1, SW], F32, tag="lrow")
                    nc.vector.tensor_copy(lA[:], avA[HD:HD + 1, :])
                    nc.vector.tensor_copy(lB[:], avB[HD:HD + 1, :])
                    rlA = smallp.tile([1, SW], F32, tag="rl")
                    rlB = smallp.tile([1, SW], F32, tag="rl")
                    nc.vector.reciprocal_approx_fast(rlA[:], lA[:])
                    nc.vector.reciprocal_approx_fast(rlB[:], lB[:])
                    rbA = smallp.tile([HD, SW], F32, tag="rb")
                    rbB = smallp.tile([HD, SW], F32, tag="rb")
                    nc.gpsimd.partition_broadcast(rbA[:], rlA[:], channels=HD)
                    nc.gpsimd.partition_broadcast(rbB[:], rlB[:], channels=HD)
                    ss = slice(s * SW, (s + 1) * SW)
                    nc.vector.tensor_tensor(out=aoT[p][0:HD, ss],
                                            in0=avA[0:HD, :], in1=rbA[:],
                                            op=MUL)
                    nc.vector.tensor_tensor(out=aoT[p][HD:P, ss],
                                            in0=avB[0:HD, :], in1=rbB[:],
                                            op=MUL)
                return run

            wts = []
            for s in range(NSTRIP):
                G = 2 * (s + 1)
                n_kt = 4 * (s + 1)
                if s == 0:
                    marks[("S0", 0)] = len(us)
                    us.append(S_unit(0, 0)); wts.append(2)
                    us.append(S_unit(0, 1)); wts.append(2)
                for g in range(G):
                    if g == 0:
                        marks[("AV0", s)] = len(us)
                    c0m = max(2 * g - 4 * s, 0) * P
                    us.append(EA_unit(s, g, n_kt))
                    wts.append(max(2 * (SW - c0m) // 128, 2))
                    nxt = g + 2
                    if nxt < G:
                        us.append(S_unit(s, nxt)); wts.append(2)
                    elif s + 1 < NSTRIP:
                        if nxt == G:
                            marks[("S0", s + 1)] = len(us)
                            us.append(S_unit(s + 1, 0)); wts.append(2)
                        elif nxt == G + 1:
                            us.append(S_unit(s + 1, 1)); wts.append(2)
                us.append(norm_unit(s)); wts.append(10)
                marks[("normdone", s)] = len(us)
            return us, marks, wts

        def interleave(a_units, b_units, weights):
            """Emit a_units (attention, ACT-heavy) with b_units (PE-heavy)
            spread between them proportionally to the a-units' expected
            PE-idle weight. b_units are (min_idx, deadline, fn): fn may only
            be emitted after a_units[min_idx - 1], and MUST be emitted
            before a_units[deadline] (prerequisite of that unit)."""
            if not a_units:
                for _, _, u in b_units:
                    u()
                return
            wtot = sum(weights)
            wcum = 0.0
            bi = 0
            for i, u in enumerate(a_units):
                while bi < len(b_units) and b_units[bi][1] <= i:
                    b_units[bi][2]()
                    bi += 1
                u()
                wcum += weights[i]
                target = int(round(len(b_units) * wcum / wtot))
                while bi < len(b_units) and bi < target \
                        and b_units[bi][0] <= i + 1:
                    b_units[bi][2]()
                    bi += 1
            while bi < len(b_units):
                b_units[bi][2]()
                bi += 1

        BIG = 10 ** 9

        # minimal prefix of qkv(0) so attention(0) strip 0 can start
        qk_unit(0, False, 0)
        qk_unit(0, True, 0)

        # precompute all pairs' attention unit lists so each pair's first two
        # score units can be emitted inside the PREVIOUS pair's stream: the
        # next pair's exps then start with no pipeline bubble at the handoff
        built = [attn_units(p) for p in range(NPAIR)]

        for p in range(NPAIR):
            a_units, marks, weights = built[p]
            lead = 0
            a_units = a_units[lead:]
            weights = weights[lead:]

            def mk(key, p=p, lead=lead):
                return max(built[p][1][key] - lead, 0)

            fill = []
            if p == 0:
                # rest of qkv(0) + all v (all pairs): strip s prereqs before
                # the prefetched S(s, 0) unit; v tiles before the strip's
                # first AV unit
                for tt in range(min(4, NT)):
                    fill.append((0, mk(("AV0", 0)), lambda tt=tt: v_unit(tt)))
                for s in range(1, NSTRIP):
                    dq = mk(("S0", s))
                    fill.append((0, dq, lambda s=s: qk_unit(0, False, s)))
                    fill.append((0, dq, lambda s=s: qk_unit(0, True, s)))
                    dv = mk(("AV0", s))
                    for tt in range(4 * s, min(4 * s + 4, NT)):
                        fill.append((0, dv, lambda tt=tt: v_unit(tt)))
                fill += [(0, BIG, u) for u in qk_units(1)]
            elif p == 1:
                fill += [(0, BIG, u) for u in qk_units(2)]
            else:
                fill += [(mk(("normdone", min(tt // 4, NSTRIP - 1))), BIG,
                          lambda tt=tt: proj_unit(tt)) for tt in range(NT)]
            if p + 1 < NPAIR:
                # next pair's first two score units, deadline-forced before
                # this pair's final units so the exp stream never drains
                # min_idx == deadline: emitting these any earlier makes this
                # pair's later S units wait on a PSUM slot that only the
                # NEXT pair's exp (queued after all of this pair's exps)
                # frees -> cross-engine ordering deadlock
                pass
            if interleave_on:
                interleave(a_units, fill, weights)
            else:
                for _, dl, u in fill:
                    if dl < BIG:
                        u()
                for u in a_units:
                    u()
                for _, dl, u in fill:
                    if dl >= BIG:
                        u()

    nc.compile()
    return nc


def make_in_maps(x, w_attn, b_attn, w_proj):
    """Shard the full inputs into per-core input maps (host side)."""
    scale = 1.0 / math.sqrt(HD)
    tri = np.where(np.arange(P)[:, None] <= np.arange(P)[None, :],
                   0.0, -1e9).astype(NPBF)
    idn = np.eye(P, dtype=NPF).astype(NPBF)
    in_maps = []
    for core in range(N_CORES):
        b, g = divmod(core, 2)
        cs = slice(g * NQK, (g + 1) * NQK)
        wq = w_attn[:, 0 * C:1 * C][:, cs] * scale
        wk = w_attn[:, 1 * C:2 * C][:, cs]
        wv = w_attn[:, 2 * C:3 * C][:, cs]
        wqkv = np.concatenate([wq, wk, wv], axis=1).astype(NPBF)
        bq = b_attn[0 * C:1 * C][cs] * scale
        bk = b_attn[1 * C:2 * C][cs]
        bqk = np.ascontiguousarray(
            np.concatenate([bq, bk]).reshape(2 * NPAIR, P).T).astype(NPF)
        bv = b_attn[2 * C:3 * C][cs].astype(NPF).reshape(1, NQK)
        wp = w_proj[g * NQK:(g + 1) * NQK, :].astype(NPBF)
        in_maps.append({
            "xt": np.ascontiguousarray(x[b].T).astype(NPBF),
            "wqkv": wqkv, "bqk": bqk, "bv": bv, "wp": wp, "tri": tri,
            "idn": idn,
        })
    return in_maps


def combine_outputs(results, b_proj):
    outs = [np.asarray(results[i]["out"], dtype=NPF) for i in range(N_CORES)]
    out = np.stack([outs[2 * b] + outs[2 * b + 1] for b in range(B)])
    return (out + b_proj[None, None, :].astype(NPF)).astype(NPF)


def kernel(x, w_attn, b_attn, w_proj, b_proj):
    x = np.asarray(x, dtype=NPF)
    w_attn = np.asarray(w_attn, dtype=NPF)
    b_attn = np.asarray(b_attn, dtype=NPF)
    w_proj = np.asarray(w_proj, dtype=NPF)
    b_proj = np.asarray(b_proj, dtype=NPF)
    if "nc" not in _CACHE:
        _CACHE["nc"] = build(T_FULL)
    nc = _CACHE["nc"]
    in_maps = make_in_maps(x, w_attn, b_attn, w_proj)
    res = run_bass_kernel_spmd(nc, in_maps, list(range(N_CORES)))
    return combine_outputs(res.results, b_proj)
